# revision 10
# baseline (speedup 1.0000x reference)
"""Trainium2 Bass kernel for nn_DenoiseEncoderTransformer (v2).

Model (reference.py): B=4, T=2048, D=128, V=64, L=12, HM=512 encoder
transformer; quadratic token embeddings -(c-x)^2/2 padded to D, plus
pos_emb; 12 pre-LN layers (single-head full attention D=128, MLP 128->512
->128 with exact gelu); scalar readout head.

Sharding: 8 cores = 4 batches x 2 sequence halves. Each core owns 1024
query rows of one batch element. Per layer the two cores of a batch
exchange K / token-major V via a 2-rank AllReduce(add) in bf16; the
partner half is recovered as (sum - local), which is rank-symmetric (the
same SPMD program runs on both cores).

v2 changes vs v1 baseline (1.02ms):
- K bias dropped (softmax row-invariant), V bias applied after softmax
  normalization (rows sum to 1), Q bias via DVE tensor_scalar. Kills the
  per-layer scalar Identity activations.
- V computed directly token-major (z1T-chunk stationary @ WvT moving):
  kills 16 PE transposes/layer and the feature-major V projection.
- LN rstd via exp(-0.5*ln(var+eps)) (natural_log_exp table set) instead
  of sqrt -> only 2 ACT_TABLE_LOADs per layer (gelu in/out).
- Scores matmuls write a bf16 PSUM tile [128,2,512] (one bank); ONE exp
  activation per key tile (N=1024) instead of two.
- Attention software-pipelined: scores(k+1) issued before u/r(k) so the
  PE never stalls on the scalar exp.
- AllReduce instead of AllGather: half the readback DMA, one DVE
  subtract per tensor-half instead of 4 full-width add/sub ops.
- Per-half LN stats (bn_stats on [128,4,128]) to shorten the serial
  Vector chain; per-half rstd so z-chunks start early.
- Next layer's weights DMA-prefetched during attention.

NOTE: sub-512B DMA descriptors are hazardous on this stack (two small
DMAs corrupt a following DMA) - all small per-partition params are
consolidated into one [128, 128] f32 "smalls" tensor loaded by a single
512B-per-partition DMA.
"""

import os
from contextlib import ExitStack

import ml_dtypes
import numpy as np

import concourse.bass as bass
import concourse.tile as tile
from concourse import bacc, mybir
from concourse.bass_utils import run_bass_kernel_spmd
from concourse.masks import make_identity

# model constants (hardcoded per problem statement)
B, T, D, V, L, HM = 4, 2048, 128, 64, 12, 512
EPS = 1e-5
SCALE = float(np.sqrt(D))

F32 = mybir.dt.float32
F32R = mybir.dt.float32r
BF16 = mybir.dt.bfloat16
I32 = mybir.dt.int32

AF = mybir.ActivationFunctionType

# column layout of the consolidated "smalls" [128, 128] f32 tensor
_C_CQ = 0              # 1 col per layer: cq                  (12)
_C_CV = L              # 1 col per layer: cv                  (12)
_C_C1 = 2 * L          # 4 cols per layer: c1 per hm-tile     (48)
_C_C2 = 6 * L          # 1 col per layer: c2                  (12)
_C_WRO = 7 * L         # 1 col: Wro^T                         (1)
_C_BRO = 7 * L + 1     # 1 col: bro broadcast                 (1)


def _build():
    """Build + compile the 8-core SPMD program."""
    n_layers = int(os.environ.get("DET_NLAYERS", str(L)))
    dump_hc = os.environ.get("DET_DUMP", "0") == "1"
    n_cores = 8
    TL = T // 2          # local residual rows per core (1024)
    NT = TL // 128       # local 128-row chunks (8)
    NKT = T // 128       # key tiles over full sequence (16)
    NLOC = TL // 128     # local key tiles (8)

    nc = bacc.Bacc("TRN2", target_bir_lowering=False, debug=False,
                   num_devices=n_cores)

    # ---- DRAM I/O ----
    xin = nc.dram_tensor("xin", [1, TL + 128], I32, kind="ExternalInput")
    pose = nc.dram_tensor("pose", [TL, D], F32, kind="ExternalInput")
    aconst = nc.dram_tensor("aconst", [3, D], F32R, kind="ExternalInput")
    wqkT = nc.dram_tensor("wqkT", [L, 2, D, D], BF16, kind="ExternalInput")
    wvT = nc.dram_tensor("wvT", [L, D, D], BF16, kind="ExternalInput")
    w1T = nc.dram_tensor("w1T", [L, D, HM], BF16, kind="ExternalInput")
    w2T = nc.dram_tensor("w2T", [L, HM, D], BF16, kind="ExternalInput")
    smalls = nc.dram_tensor("smalls", [D, 128], F32, kind="ExternalInput")
    out = nc.dram_tensor("out", [1, TL], F32, kind="ExternalOutput")
    if dump_hc:
        dbg = nc.dram_tensor("dbg", [128, NT, 128], F32,
                             kind="ExternalOutput")

    with tile.TileContext(nc) as tc, ExitStack() as ctx:
        # ---- pools ----
        singles = ctx.enter_context(tc.tile_pool(name="singles", bufs=1))
        persist = ctx.enter_context(tc.tile_pool(name="persist", bufs=1))
        params = ctx.enter_context(tc.tile_pool(name="params", bufs=2))
        work = ctx.enter_context(tc.tile_pool(name="work", bufs=1))
        et_pool = ctx.enter_context(tc.tile_pool(name="et", bufs=4))
        small = ctx.enter_context(tc.tile_pool(name="small", bufs=4))
        ps_gen = ctx.enter_context(
            tc.tile_pool(name="ps_gen", bufs=2, space="PSUM"))
        ps_acc = ctx.enter_context(
            tc.tile_pool(name="ps_acc", bufs=4, space="PSUM"))
        dram = ctx.enter_context(
            tc.tile_pool(name="dram", bufs=2, space="DRAM"))

        # ---- one-time setup ----
        ident_f = singles.tile([128, 128], F32)
        make_identity(nc, ident_f)
        ident = singles.tile([128, 128], F32R)
        nc.scalar.copy(ident, ident_f)
        ident_b = singles.tile([128, 128], BF16)
        nc.scalar.copy(ident_b, ident_f)
        ones_f = singles.tile([128, 128], F32)
        nc.gpsimd.memset(ones_f, 1.0)
        ones_t = singles.tile([128, 128], BF16)
        nc.scalar.copy(ones_t, ones_f)
        eps_t = singles.tile([128, 1], F32)
        nc.vector.memset(eps_t, EPS)

        sm = singles.tile([D, 128], F32)
        nc.sync.dma_start(sm, smalls[:, :])
        acon_s = singles.tile([3, D], F32R)
        nc.sync.dma_start(acon_s, aconst[:, :])

        # ---- embedding: Hc[t, d] = -(c - x)^2/2 (padded) + pos_emb ----
        x_i = singles.tile([1, TL + 128], I32)
        nc.sync.dma_start(x_i, xin[:, :])
        x_f = singles.tile([1, TL], F32)
        nc.vector.tensor_copy(x_f, x_i[:, 0:TL])
        x_sq = singles.tile([1, TL], F32)
        nc.scalar.square(x_sq, x_f)
        xf_f = singles.tile([3, TL], F32)
        nc.vector.memset(xf_f, 1.0)
        nc.sync.dma_start(xf_f[1:2, :], x_f)
        nc.sync.dma_start(xf_f[2:3, :], x_sq)
        xf = singles.tile([3, TL], F32R)
        nc.scalar.copy(xf, xf_f)

        Hc = persist.tile([128, NT, 128], F32)  # residual, token-major
        pos_s = work.tile([128, NT, 128], F32)
        nc.sync.dma_start(pos_s, pose.rearrange("(n p) d -> p n d", p=128))
        for i in range(NT):
            pse = ps_gen.tile([128, 128], F32, tag="psg")
            nc.tensor.matmul(pse, xf[:, bass.ts(i, 128)], acon_s,
                             start=True, stop=True)
            nc.vector.tensor_add(Hc[:, i, :], pse, pos_s[:, i, :])

        # ---- per-layer weight prefetch (params pool, bufs=2) ----
        wtiles = {}

        def prefetch(l):
            if l >= n_layers:
                return
            wqk_s = params.tile([128, 2, 128], BF16, tag="wqk",
                                name=f"wqk{l}")
            nc.sync.dma_start(wqk_s, wqkT[l].rearrange("a p d -> p a d"))
            wv_s = params.tile([128, 128], BF16, tag="wv", name=f"wv{l}")
            nc.sync.dma_start(wv_s, wvT[l])
            w1_s = params.tile([128, HM], BF16, tag="w1", name=f"w1{l}")
            nc.sync.dma_start(w1_s, w1T[l])
            w2_s = params.tile([128, HM // 128, 128], BF16, tag="w2",
                               name=f"w2{l}")
            nc.sync.dma_start(
                w2_s, w2T[l].rearrange("(m p) d -> p m d", p=128))
            wtiles[l] = (wqk_s, wv_s, w1_s, w2_s)

        prefetch(0)

        def layernorm_T(src_chunks, dst, tagp):
            """LN(src) per 128-row chunk -> transpose -> dst [d, TL] bf16.

            Stats are computed per 4-chunk half so the first z chunks can
            start while the second half's stats are still running.
            """
            mv2 = small.tile([128, NT, 2], F32, tag=f"mv{tagp}")
            rstd = small.tile([128, NT], F32, tag=f"rs{tagp}")
            for h in range(2):
                c0 = h * 4
                for i in range(4):
                    st = small.tile([128, 6], F32, tag=f"st{tagp}")
                    nc.vector.bn_stats(st, src_chunks[:, c0 + i, :])
                    nc.vector.bn_aggr(mv2[:, c0 + i, :], st)
                lnv = small.tile([128, 4], F32, tag=f"ln{tagp}")
                nc.scalar.activation(lnv, mv2[:, c0:c0 + 4, 1], AF.Ln,
                                     bias=eps_t, scale=1.0)
                nc.scalar.activation(rstd[:, c0:c0 + 4], lnv, AF.Exp,
                                     scale=-0.5)
                for i in range(c0, c0 + 4):
                    z = small.tile([128, 128], BF16, tag=f"z{tagp}")
                    nc.vector.tensor_scalar(
                        out=z, in0=src_chunks[:, i, :],
                        scalar1=mv2[:, i, 0:1], scalar2=rstd[:, i:i + 1],
                        op0=mybir.AluOpType.subtract,
                        op1=mybir.AluOpType.mult)
                    pst = ps_gen.tile([128, 128], BF16, tag="psg")
                    nc.tensor.transpose(pst, z, ident_b)
                    nc.vector.tensor_copy(dst[:, bass.ts(i, 128)], pst)

        for l in range(n_layers):
            wqk_s, wv_s, w1_s, w2_s = wtiles[l]

            # ---- LN1 + transpose ----
            z1T = work.tile([128, TL], BF16, tag="zT")
            layernorm_T(Hc, z1T, "a")

            # ---- K projection (no bias; softmax-invariant) ----
            kps = ps_gen.tile([128, 2, 512], F32, tag="psg")
            for j in range(2):
                nc.tensor.matmul(kps[:, j, :], wqk_s[:, 1, :],
                                 z1T[:, bass.ds(j * 512, 512)],
                                 start=True, stop=True)
            KTl = work.tile([128, TL], BF16, tag="KTl")
            nc.scalar.copy(KTl, kps.rearrange("p a b -> p (a b)"))
            cc_in = dram.tile([2, 128, TL], BF16, tag="cci")
            nc.sync.dma_start(cc_in[0], KTl)

            # ---- V projection, directly token-major (no bias) ----
            Vtok = work.tile([128, NLOC, 128], BF16, tag="Vtok")
            for k in range(NLOC):
                vps = ps_gen.tile([128, 128], F32, tag="psg")
                nc.tensor.matmul(vps, z1T[:, bass.ts(k, 128)], wv_s,
                                 start=True, stop=True)
                nc.vector.tensor_copy(Vtok[:, k, :], vps)
            nc.sync.dma_start(
                cc_in[1], Vtok.rearrange("p a b -> p (a b)"))

            # ---- K/V sum exchange (2-rank AllReduce, bf16) ----
            cc_out = dram.tile([2, 128, TL], BF16, tag="cco")
            nc.gpsimd.collective_compute(
                "AllReduce", mybir.AluOpType.add,
                ins=[cc_in[:].opt()], outs=[cc_out[:].opt()],
                replica_groups=[[0, 1], [2, 3], [4, 5], [6, 7]],
            )
            Ksum = work.tile([128, TL], BF16, tag="Ksum")
            nc.sync.dma_start(Ksum, cc_out[0])
            Vsum = work.tile([128, TL], BF16, tag="Vsum")
            nc.sync.dma_start(Vsum, cc_out[1])

            # prefetch next layer's weights while attention runs
            prefetch(l + 1)

            # ---- Q projection (+bias cq via DVE) ----
            qps = ps_gen.tile([128, 2, 512], F32, tag="psg")
            for j in range(2):
                nc.tensor.matmul(qps[:, j, :], wqk_s[:, 0, :],
                                 z1T[:, bass.ds(j * 512, 512)],
                                 start=True, stop=True)
            QT = work.tile([128, 2, 512], BF16, tag="QT")
            cq_ap = sm[:, _C_CQ + l:_C_CQ + l + 1]
            for j in range(2):
                nc.vector.tensor_scalar_add(QT[:, j, :], qps[:, j, :], cq_ap)

            # ---- partner K/V = sum - local (issued early; V idle during
            #      attention, so these fire as soon as the DMAs land) ----
            KTr = work.tile([128, TL], BF16, tag="KTr")
            Vtr = work.tile([128, NLOC, 128], BF16, tag="Vtr")
            for j in range(2):
                sl = bass.ds(j * 512, 512)
                nc.vector.tensor_tensor(
                    out=KTr[:, sl], in0=Ksum[:, sl], in1=KTl[:, sl],
                    op=mybir.AluOpType.subtract)
            vtr_flat = Vtr.rearrange("p a b -> p (a b)")
            vtok_flat = Vtok.rearrange("p a b -> p (a b)")
            for j in range(2):
                sl = bass.ds(j * 512, 512)
                nc.vector.tensor_tensor(
                    out=vtr_flat[:, sl], in0=Vsum[:, sl],
                    in1=vtok_flat[:, sl], op=mybir.AluOpType.subtract)

            # ---- attention: software-pipelined over 16 key tiles ----
            accs = [ps_acc.tile([128, 512], F32, tag="psa",
                                name=f"acc{l}_{i}") for i in range(4)]
            uL, uR, rL, rR = accs

            def kt_ap(k):
                if k < NLOC:
                    return KTl[:, bass.ts(k, 128)]
                return KTr[:, bass.ts(k - NLOC, 128)]

            def vt_ap(k):
                if k < NLOC:
                    return Vtok[:, k, :]
                return Vtr[:, k - NLOC, :]

            def scores(k):
                sp = ps_gen.tile([128, 2, 512], F32, tag="psg",
                                 name=f"sp{l}_{k}")
                for j in range(2):
                    nc.tensor.matmul(sp[:, j, :], kt_ap(k),
                                     QT[:, j, :], start=True, stop=True)
                return sp

            sp_cur = scores(0)
            for k in range(NKT):
                first, last = (k == 0), (k == NKT - 1)
                et = et_pool.tile([128, 2, 512], BF16, tag="et")
                nc.scalar.activation(et, sp_cur, AF.Exp, scale=1.0 / SCALE)
                if k + 1 < NKT:
                    sp_cur = scores(k + 1)
                nc.tensor.matmul(uL, vt_ap(k), et[:, 0, :],
                                 start=first, stop=last)
                nc.tensor.matmul(uR, vt_ap(k), et[:, 1, :],
                                 start=first, stop=last)
                nc.tensor.matmul(rL, ones_t, et[:, 0, :],
                                 start=first, stop=last)
                nc.tensor.matmul(rR, ones_t, et[:, 1, :],
                                 start=first, stop=last)

            # ---- normalize (u/r), +cv, transpose back, residual add ----
            cv_ap = sm[:, _C_CV + l:_C_CV + l + 1]
            UTn = work.tile([128, 2, 512], BF16, tag="UTn")
            for (u_ps, r_ps, j) in ((uL, rL, 0), (uR, rR, 1)):
                rrec = work.tile([128, 512], F32, tag="rrec")
                nc.vector.reciprocal_approx_fast(rrec, r_ps)
                ut = work.tile([128, 512], BF16, tag="ut")
                nc.vector.tensor_mul(ut, u_ps, rrec)
                nc.vector.tensor_scalar_add(UTn[:, j, :], ut, cv_ap)
            utn_flat = UTn.rearrange("p a b -> p (a b)")
            for i in range(NT):
                psu = ps_gen.tile([128, 128], BF16, tag="psg")
                nc.tensor.transpose(psu, utn_flat[:, bass.ts(i, 128)],
                                    ident_b)
                nc.vector.tensor_add(Hc[:, i, :], psu, Hc[:, i, :])

            # ---- LN2 + transpose ----
            z2T = work.tile([128, TL], BF16, tag="zT")
            layernorm_T(Hc, z2T, "b")

            # ---- MLP: h1 chunks + gelu interleaved, then W2 per half ----
            gts = []
            for m in range(HM // 128):
                hp = ps_gen.tile([128, 2, 512], F32, tag="psg",
                                 name=f"hp{l}_{m}")
                for j in range(2):
                    nc.tensor.matmul(hp[:, j, :], w1_s[:, bass.ts(m, 128)],
                                     z2T[:, bass.ds(j * 512, 512)],
                                     start=True, stop=True)
                gt = et_pool.tile([128, 2, 512], BF16, tag="et",
                                  name=f"gt{l}_{m}")
                c1_ap = sm[:, _C_C1 + 4 * l + m:_C_C1 + 4 * l + m + 1]
                nc.scalar.activation(gt, hp, AF.Gelu, bias=c1_ap, scale=1.0)
                gts.append(gt)
            OT = work.tile([128, 2, 512], BF16, tag="OT")
            c2_ap = sm[:, _C_C2 + l:_C_C2 + l + 1]
            for j in range(2):
                pso = ps_acc.tile([128, 512], F32, tag="psa")
                for m in range(HM // 128):
                    nc.tensor.matmul(pso, w2_s[:, m, :], gts[m][:, j, :],
                                     start=(m == 0),
                                     stop=(m == HM // 128 - 1))
                nc.vector.tensor_scalar_add(OT[:, j, :], pso, c2_ap)
            ot_flat = OT.rearrange("p a b -> p (a b)")
            for i in range(NT):
                pst = ps_gen.tile([128, 128], BF16, tag="psg")
                nc.tensor.transpose(pst, ot_flat[:, bass.ts(i, 128)],
                                    ident_b)
                nc.vector.tensor_add(Hc[:, i, :], pst, Hc[:, i, :])

        if dump_hc:
            nc.sync.dma_start(dbg[:, :, :], Hc)

        # ---- head: pred = Hc @ Wro^T + bro ----
        wro_s = singles.tile([128, 1], F32R)
        nc.scalar.copy(wro_s, sm[:, _C_WRO:_C_WRO + 1])
        HcT = work.tile([128, TL], F32R, tag="hct")
        for i in range(NT):
            hr = small.tile([128, 128], F32R, tag="hr")
            nc.vector.tensor_copy(hr, Hc[:, i, :])
            psh = ps_gen.tile([128, 128], F32R, tag="psg")
            nc.tensor.transpose(psh, hr, ident)
            nc.vector.tensor_copy(HcT[:, bass.ts(i, 128)], psh)
        pred = work.tile([1, TL], F32, tag="pred")
        for j in range(TL // 512):
            psp = ps_gen.tile([1, 512], F32, tag="psg")
            nc.tensor.matmul(psp, wro_s, HcT[:, bass.ts(j, 512)],
                             start=True, stop=True)
            nc.scalar.activation(pred[:, bass.ts(j, 512)], psp,
                                 AF.Identity,
                                 bias=sm[0:1, _C_BRO:_C_BRO + 1], scale=1.0)
        nc.sync.dma_start(out[:, :], pred)

    nc.compile()
    return nc, n_cores


_CACHE = {}


def _get_built(mode="m8"):
    if "m8" not in _CACHE:
        _CACHE["m8"] = _build()
    return _CACHE["m8"]


MODE = "m8"


def _prep_inputs(mode, x, pos_emb, Wq, Wk, Wv, ln1_g, ln1_b, W1, b1, W2, b2,
                 ln2_g, ln2_b, Wro, bro):
    """Host-side shard + fold. Returns list of per-core input dicts."""
    n_cores = 8
    TL = T // 2

    f32 = np.float32
    x = np.asarray(x).astype(np.int32)
    pos_emb = np.asarray(pos_emb, f32)
    Wq, Wk, Wv = (np.asarray(a, f32) for a in (Wq, Wk, Wv))
    ln1_g, ln1_b = np.asarray(ln1_g, f32), np.asarray(ln1_b, f32)
    W1, b1 = np.asarray(W1, f32), np.asarray(b1, f32)
    W2, b2 = np.asarray(W2, f32), np.asarray(b2, f32)
    ln2_g, ln2_b = np.asarray(ln2_g, f32), np.asarray(ln2_b, f32)
    Wro, bro = np.asarray(Wro, f32), np.asarray(bro, f32)

    # fold LN gains into projections; LN betas into the bias columns
    wqkT = np.stack([
        np.stack([(Wq[l] * ln1_g[l][None, :]).T,
                  (Wk[l] * ln1_g[l][None, :]).T]) for l in range(L)])
    wvT = np.stack([(Wv[l] * ln1_g[l][None, :]).T for l in range(L)])
    w1T = np.stack([(W1[l] * ln2_g[l][None, :]).T for l in range(L)])
    w2T = np.stack([W2[l].T for l in range(L)])

    smalls = np.zeros((D, 128), f32)
    for l in range(L):
        smalls[:, _C_CQ + l] = ln1_b[l] @ Wq[l].T
        smalls[:, _C_CV + l] = ln1_b[l] @ Wv[l].T
        c1 = (b1[l] + ln2_b[l] @ W1[l].T).reshape(HM // D, D)
        for m in range(HM // D):
            smalls[:, _C_C1 + 4 * l + m] = c1[m]
        smalls[:, _C_C2 + l] = b2[l]
    smalls[:, _C_WRO] = Wro[0]
    smalls[:, _C_BRO] = float(bro.reshape(-1)[0])

    ar = np.arange(D, dtype=f32)
    mask = (ar < V).astype(f32)
    aconst = np.stack([-0.5 * ar * ar * mask, ar * mask, -0.5 * mask])

    bf16 = ml_dtypes.bfloat16
    common = dict(
        aconst=np.ascontiguousarray(aconst, f32),
        wqkT=np.ascontiguousarray(wqkT.astype(bf16)),
        wvT=np.ascontiguousarray(wvT.astype(bf16)),
        w1T=np.ascontiguousarray(w1T.astype(bf16)),
        w2T=np.ascontiguousarray(w2T.astype(bf16)),
        smalls=np.ascontiguousarray(smalls, f32),
    )
    in_maps = []
    for c in range(n_cores):
        b_idx, half = c // 2, c % 2
        sl = slice(half * TL, (half + 1) * TL)
        xrow = np.zeros((1, TL + 128), np.int32)
        xrow[0, :TL] = x[b_idx, sl]
        in_maps.append(dict(
            common,
            xin=xrow,
            pose=np.ascontiguousarray(pos_emb[sl]),
        ))
    return in_maps


def kernel(**inputs):
    nc, n_cores = _get_built()
    in_maps = _prep_inputs("m8", **inputs)
    res = run_bass_kernel_spmd(nc, in_maps, core_ids=list(range(n_cores)))
    TL = T // 2
    out = np.empty((B, T), np.float32)
    for c in range(n_cores):
        b_idx, half = c // 2, c % 2
        out[b_idx, half * TL:(half + 1) * TL] = res.results[c]["out"][0]
    return out


# revision 12
# speedup vs baseline: 1.2651x; 1.2651x over previous
"""Trainium2 Bass kernel for nn_DenoiseEncoderTransformer (v3).

Model (reference.py): B=4, T=2048, D=128, V=64, L=12, HM=512 encoder
transformer; quadratic token embeddings -(c-x)^2/2 padded to D, plus
pos_emb; 12 pre-LN layers (single-head full attention D=128, MLP 128->512
->128 with exact gelu); scalar readout head.

Sharding: 8 cores = 4 batches x 2 sequence halves. Each core owns 1024
query rows of one batch element. Per layer the two cores of a batch
exchange the LN1 output z1T via a 2-rank bf16 AllGather (started right
after LN1, ~2.5us earlier and half the payload of a K/V exchange); the
partner half is recovered as (g0+g1) - local (rank-symmetric for the
shared SPMD program), and the remote K / token-major V are recomputed
locally from the partner z (10 extra small matmuls, fully overlapped).

Other structure (see v2 notes): K bias dropped (softmax row-invariant),
V bias applied post-normalization, Q bias via DVE; V computed directly
token-major; attention software-pipelined 2 tiles ahead with 4
single-bank score PSUM tiles and per-half exps so the PE never waits on
the scalar engine; MLP W2 accumulation interleaved with the gelus;
next-layer weights prefetched before the collective is issued.

NOTE: sub-512B DMA descriptors are hazardous on this stack (two small
DMAs corrupt a following DMA) - all small per-partition params are
consolidated into one [128, 128] f32 "smalls" tensor loaded by a single
512B-per-partition DMA.
"""

import os
from contextlib import ExitStack

import ml_dtypes
import numpy as np

import concourse.bass as bass
import concourse.tile as tile
from concourse import bacc, mybir
from concourse.bass_utils import run_bass_kernel_spmd
from concourse.masks import make_identity

# model constants (hardcoded per problem statement)
B, T, D, V, L, HM = 4, 2048, 128, 64, 12, 512
EPS = 1e-5
SCALE = float(np.sqrt(D))

F32 = mybir.dt.float32
F32R = mybir.dt.float32r
BF16 = mybir.dt.bfloat16
I32 = mybir.dt.int32

AF = mybir.ActivationFunctionType

# column layout of the consolidated "smalls" [128, 128] f32 tensor
_C_CQ = 0              # 1 col per layer: cq                  (12)
_C_CV = L              # 1 col per layer: cv                  (12)
_C_C1 = 2 * L          # 4 cols per layer: c1 per hm-tile     (48)
_C_C2 = 6 * L          # 1 col per layer: c2                  (12)
_C_WRO = 7 * L         # 1 col: Wro^T                         (1)
_C_BRO = 7 * L + 1     # 1 col: bro broadcast                 (1)


def _build():
    """Build + compile the 8-core SPMD program."""
    n_layers = int(os.environ.get("DET_NLAYERS", str(L)))
    dump_hc = os.environ.get("DET_DUMP", "0") == "1"
    n_cores = 8
    TL = T // 2          # local residual rows per core (1024)
    NT = TL // 128       # local 128-row chunks (8)
    NKT = T // 128       # key tiles over full sequence (16)
    NLOC = TL // 128     # local key tiles (8)

    nc = bacc.Bacc("TRN2", target_bir_lowering=False, debug=False,
                   num_devices=n_cores)

    # ---- DRAM I/O ----
    xin = nc.dram_tensor("xin", [1, TL + 128], I32, kind="ExternalInput")
    pose = nc.dram_tensor("pose", [TL, D], F32, kind="ExternalInput")
    aconst = nc.dram_tensor("aconst", [3, D], F32R, kind="ExternalInput")
    wqkT = nc.dram_tensor("wqkT", [L, 2, D, D], BF16, kind="ExternalInput")
    wvT = nc.dram_tensor("wvT", [L, D, D], BF16, kind="ExternalInput")
    w1T = nc.dram_tensor("w1T", [L, D, HM], BF16, kind="ExternalInput")
    w2T = nc.dram_tensor("w2T", [L, HM, D], BF16, kind="ExternalInput")
    smalls = nc.dram_tensor("smalls", [D, 128], F32, kind="ExternalInput")
    out = nc.dram_tensor("out", [1, TL], F32, kind="ExternalOutput")
    if dump_hc:
        dbg = nc.dram_tensor("dbg", [128, NT, 128], F32,
                             kind="ExternalOutput")

    with tile.TileContext(nc) as tc, ExitStack() as ctx:
        # ---- pools ----
        singles = ctx.enter_context(tc.tile_pool(name="singles", bufs=1))
        persist = ctx.enter_context(tc.tile_pool(name="persist", bufs=1))
        params = ctx.enter_context(tc.tile_pool(name="params", bufs=2))
        work = ctx.enter_context(tc.tile_pool(name="work", bufs=1))
        et_pool = ctx.enter_context(tc.tile_pool(name="et", bufs=4))
        small = ctx.enter_context(tc.tile_pool(name="small", bufs=4))
        # PSUM: 4 single-bank score tiles + 4 accumulators = 8 banks
        ps_sc = ctx.enter_context(
            tc.tile_pool(name="ps_sc", bufs=4, space="PSUM"))
        ps_acc = ctx.enter_context(
            tc.tile_pool(name="ps_acc", bufs=4, space="PSUM"))
        dram = ctx.enter_context(
            tc.tile_pool(name="dram", bufs=2, space="DRAM"))

        # ---- one-time setup ----
        ident_f = singles.tile([128, 128], F32)
        make_identity(nc, ident_f)
        ident = singles.tile([128, 128], F32R)
        nc.scalar.copy(ident, ident_f)
        ident_b = singles.tile([128, 128], BF16)
        nc.scalar.copy(ident_b, ident_f)
        ones_f = singles.tile([128, 128], F32)
        nc.gpsimd.memset(ones_f, 1.0)
        ones_t = singles.tile([128, 128], BF16)
        nc.scalar.copy(ones_t, ones_f)
        eps_t = singles.tile([128, 1], F32)
        nc.vector.memset(eps_t, EPS)

        sm = singles.tile([D, 128], F32)
        nc.sync.dma_start(sm, smalls[:, :])
        acon_s = singles.tile([3, D], F32R)
        nc.sync.dma_start(acon_s, aconst[:, :])

        # ---- embedding: Hc[t, d] = -(c - x)^2/2 (padded) + pos_emb ----
        x_i = singles.tile([1, TL + 128], I32)
        nc.sync.dma_start(x_i, xin[:, :])
        x_f = singles.tile([1, TL], F32)
        nc.vector.tensor_copy(x_f, x_i[:, 0:TL])
        x_sq = singles.tile([1, TL], F32)
        nc.scalar.square(x_sq, x_f)
        xf_f = singles.tile([3, TL], F32)
        nc.vector.memset(xf_f, 1.0)
        nc.sync.dma_start(xf_f[1:2, :], x_f)
        nc.sync.dma_start(xf_f[2:3, :], x_sq)
        xf = singles.tile([3, TL], F32R)
        nc.scalar.copy(xf, xf_f)

        Hc = persist.tile([128, NT, 128], F32)  # residual, token-major
        pos_s = work.tile([128, NT, 128], F32)
        nc.sync.dma_start(pos_s, pose.rearrange("(n p) d -> p n d", p=128))
        for i in range(NT):
            pse = ps_sc.tile([128, 512], F32, tag="psg")
            nc.tensor.matmul(pse[:, 0:128], xf[:, bass.ts(i, 128)], acon_s,
                             start=True, stop=True)
            nc.vector.tensor_add(Hc[:, i, :], pse[:, 0:128], pos_s[:, i, :])

        # ---- per-layer weight prefetch (params pool, bufs=2) ----
        wtiles = {}

        def prefetch(l):
            if l >= n_layers or l in wtiles:
                return
            wqk_s = params.tile([128, 2, 128], BF16, tag="wqk",
                                name=f"wqk{l}")
            nc.sync.dma_start(wqk_s, wqkT[l].rearrange("a p d -> p a d"))
            wv_s = params.tile([128, 128], BF16, tag="wv", name=f"wv{l}")
            nc.sync.dma_start(wv_s, wvT[l])
            w1_s = params.tile([128, HM], BF16, tag="w1", name=f"w1{l}")
            nc.sync.dma_start(w1_s, w1T[l])
            w2_s = params.tile([128, HM // 128, 128], BF16, tag="w2",
                               name=f"w2{l}")
            nc.sync.dma_start(
                w2_s, w2T[l].rearrange("(m p) d -> p m d", p=128))
            wtiles[l] = (wqk_s, wv_s, w1_s, w2_s)

        prefetch(0)

        def layernorm_T(src_chunks, dst, tagp):
            """LN(src) per 128-row chunk -> transpose -> dst [d, TL] bf16.

            Stats per 4-chunk half so the first z chunks start while the
            second half's stats are still running.
            """
            mv2 = small.tile([128, NT, 2], F32, tag=f"mv{tagp}")
            rstd = small.tile([128, NT], F32, tag=f"rs{tagp}")
            for h in range(2):
                c0 = h * 4
                for i in range(4):
                    st = small.tile([128, 6], F32, tag=f"st{tagp}")
                    nc.vector.bn_stats(st, src_chunks[:, c0 + i, :])
                    nc.vector.bn_aggr(mv2[:, c0 + i, :], st)
                sq = small.tile([128, 4], F32, tag=f"sq{tagp}")
                nc.scalar.activation(sq, mv2[:, c0:c0 + 4, 1], AF.Sqrt,
                                     bias=eps_t, scale=1.0)
                nc.vector.reciprocal(rstd[:, c0:c0 + 4], sq)
                for i in range(c0, c0 + 4):
                    z = small.tile([128, 128], BF16, tag=f"z{tagp}")
                    nc.vector.tensor_scalar(
                        out=z, in0=src_chunks[:, i, :],
                        scalar1=mv2[:, i, 0:1], scalar2=rstd[:, i:i + 1],
                        op0=mybir.AluOpType.subtract,
                        op1=mybir.AluOpType.mult)
                    pst = ps_sc.tile([128, 512], BF16, tag="psg")
                    nc.tensor.transpose(pst[:, 0:128], z, ident_b)
                    nc.vector.tensor_copy(dst[:, bass.ts(i, 128)],
                                          pst[:, 0:128])

        for l in range(n_layers):
            wqk_s, wv_s, w1_s, w2_s = wtiles[l]

            # ---- LN1 -> z1T; start the z exchange immediately ----
            z1T = work.tile([128, TL], BF16, tag="zT")
            layernorm_T(Hc, z1T, "a")
            cc_in = dram.tile([128, TL], BF16, tag="cci")
            nc.sync.dma_start(cc_in, z1T)
            prefetch(l + 1)
            cc_out = dram.tile([2, 128, TL], BF16, tag="cco")
            nc.gpsimd.collective_compute(
                "AllGather", mybir.AluOpType.bypass,
                ins=[cc_in[:].opt()], outs=[cc_out[:].opt()],
                replica_groups=[[0, 1], [2, 3], [4, 5], [6, 7]],
            )
            g0 = work.tile([128, TL], BF16, tag="g0")
            nc.sync.dma_start(g0, cc_out[0])
            g1 = work.tile([128, TL], BF16, tag="g1")
            nc.sync.dma_start(g1, cc_out[1])

            # ---- local K / V / Q projections off z1T ----
            KTl = work.tile([128, TL], BF16, tag="KTl")
            for j in range(2):
                kps = ps_acc.tile([128, 512], F32, tag="psa",
                                  name=f"kps{l}_{j}")
                nc.tensor.matmul(kps, wqk_s[:, 1, :],
                                 z1T[:, bass.ds(j * 512, 512)],
                                 start=True, stop=True)
                nc.scalar.copy(KTl[:, bass.ds(j * 512, 512)], kps)

            Vtok = work.tile([128, NLOC, 128], BF16, tag="Vtok")
            for k in range(NLOC):
                vps = ps_sc.tile([128, 512], F32, tag="psg")
                nc.tensor.matmul(vps[:, 0:128], z1T[:, bass.ts(k, 128)],
                                 wv_s, start=True, stop=True)
                nc.vector.tensor_copy(Vtok[:, k, :], vps[:, 0:128])

            QT = work.tile([128, 2, 512], BF16, tag="QT")
            cq_ap = sm[:, _C_CQ + l:_C_CQ + l + 1]
            for j in range(2):
                qps = ps_acc.tile([128, 512], F32, tag="psa",
                                  name=f"qps{l}_{j}")
                nc.tensor.matmul(qps, wqk_s[:, 0, :],
                                 z1T[:, bass.ds(j * 512, 512)],
                                 start=True, stop=True)
                nc.vector.tensor_scalar_add(QT[:, j, :], qps, cq_ap)

            # ---- partner z = (g0+g1) - local, per half (on Vector; idle
            #      during attention so these fire as soon as DMAs land) ----
            zrT = work.tile([128, TL], BF16, tag="zrT")
            for j in range(2):
                sl = bass.ds(j * 512, 512)
                gs = work.tile([128, 512], BF16, tag="gs", name=f"gs{l}_{j}")
                nc.vector.tensor_add(gs, g0[:, sl], g1[:, sl])
                nc.vector.tensor_tensor(
                    out=zrT[:, sl], in0=gs, in1=z1T[:, sl],
                    op=mybir.AluOpType.subtract)

            # ---- attention: 2-ahead pipeline over 16 key tiles ----
            accs = [ps_acc.tile([128, 512], F32, tag="psa",
                                name=f"acc{l}_{i}") for i in range(4)]
            uL, uR, rL, rR = accs
            KTr = work.tile([128, TL], BF16, tag="KTr")
            Vtr = work.tile([128, NLOC, 128], BF16, tag="Vtr")

            def kt_ap(k):
                if k < NLOC:
                    return KTl[:, bass.ts(k, 128)]
                return KTr[:, bass.ts(k - NLOC, 128)]

            def vt_ap(k):
                if k < NLOC:
                    return Vtok[:, k, :]
                return Vtr[:, k - NLOC, :]

            def scores_half(k, j):
                sp = ps_sc.tile([128, 512], F32, tag="psg",
                                name=f"sp{l}_{k}_{j}")
                nc.tensor.matmul(sp, kt_ap(k), QT[:, j, :],
                                 start=True, stop=True)
                return sp

            def kr_proj(j):
                """Remote K half j from partner z."""
                krp = ps_sc.tile([128, 512], F32, tag="psg")
                nc.tensor.matmul(krp, wqk_s[:, 1, :],
                                 zrT[:, bass.ds(j * 512, 512)],
                                 start=True, stop=True)
                nc.vector.tensor_copy(KTr[:, bass.ds(j * 512, 512)], krp)

            def vr_proj(k):
                """Remote token-major V chunk k from partner z."""
                vrp = ps_sc.tile([128, 512], F32, tag="psg")
                nc.tensor.matmul(vrp[:, 0:128], zrT[:, bass.ts(k, 128)],
                                 wv_s, start=True, stop=True)
                nc.vector.tensor_copy(Vtr[:, k, :], vrp[:, 0:128])

            sps = {}
            for (k, j) in ((0, 0), (0, 1), (1, 0), (1, 1)):
                sps[(k, j)] = scores_half(k, j)
            for k in range(NKT):
                first, last = (k == 0), (k == NKT - 1)
                et = et_pool.tile([128, 2, 512], BF16, tag="et")
                for j in range(2):
                    nc.scalar.activation(et[:, j, :], sps.pop((k, j)),
                                         AF.Exp, scale=1.0 / SCALE)
                # remote K/V projections injected mid-pipeline: kr before
                # the k=8 scores issue (at k==6), vr before the k=8 u's
                if k == 6:
                    kr_proj(0)
                    kr_proj(1)
                elif k == 7:
                    for kk in range(0, 4):
                        vr_proj(kk)
                elif k == 8:
                    for kk in range(4, 8):
                        vr_proj(kk)
                if k + 2 < NKT:
                    for j in range(2):
                        sps[(k + 2, j)] = scores_half(k + 2, j)
                nc.tensor.matmul(uL, vt_ap(k), et[:, 0, :],
                                 start=first, stop=last)
                nc.tensor.matmul(uR, vt_ap(k), et[:, 1, :],
                                 start=first, stop=last)
                nc.tensor.matmul(rL, ones_t, et[:, 0, :],
                                 start=first, stop=last)
                nc.tensor.matmul(rR, ones_t, et[:, 1, :],
                                 start=first, stop=last)

            # ---- normalize (u/r), +cv, transpose back, residual add ----
            cv_ap = sm[:, _C_CV + l:_C_CV + l + 1]
            UTn = work.tile([128, 2, 512], BF16, tag="UTn")
            for (u_ps, r_ps, j) in ((uL, rL, 0), (uR, rR, 1)):
                rrec = work.tile([128, 512], F32, tag="rrec")
                nc.vector.reciprocal_approx_fast(rrec, r_ps)
                ut = work.tile([128, 512], BF16, tag="ut")
                nc.vector.tensor_mul(ut, u_ps, rrec)
                nc.vector.tensor_scalar_add(UTn[:, j, :], ut, cv_ap)
            utn_flat = UTn.rearrange("p a b -> p (a b)")
            for i in range(NT):
                psu = ps_sc.tile([128, 512], BF16, tag="psg")
                nc.tensor.transpose(psu[:, 0:128],
                                    utn_flat[:, bass.ts(i, 128)], ident_b)
                nc.vector.tensor_add(Hc[:, i, :], psu[:, 0:128], Hc[:, i, :])

            # ---- LN2 + transpose ----
            z2T = work.tile([128, TL], BF16, tag="zT")
            layernorm_T(Hc, z2T, "b")

            # ---- MLP: W2-halves interleaved with the per-half gelus ----
            psoL = ps_acc.tile([128, 512], F32, tag="psa")
            psoR = ps_acc.tile([128, 512], F32, tag="psa")
            gts = []
            for m in range(HM // 128):
                gt = et_pool.tile([128, 2, 512], BF16, tag="et",
                                  name=f"gt{l}_{m}")
                c1_ap = sm[:, _C_C1 + 4 * l + m:_C_C1 + 4 * l + m + 1]
                for j in range(2):
                    hp = ps_sc.tile([128, 512], F32, tag="psg",
                                    name=f"hp{l}_{m}_{j}")
                    nc.tensor.matmul(hp, w1_s[:, bass.ts(m, 128)],
                                     z2T[:, bass.ds(j * 512, 512)],
                                     start=True, stop=True)
                    nc.scalar.activation(gt[:, j, :], hp, AF.Gelu,
                                         bias=c1_ap, scale=1.0)
                gts.append(gt)
                nc.tensor.matmul(psoL, w2_s[:, m, :], gt[:, 0, :],
                                 start=(m == 0), stop=(m == HM // 128 - 1))
            for m in range(HM // 128):
                nc.tensor.matmul(psoR, w2_s[:, m, :], gts[m][:, 1, :],
                                 start=(m == 0), stop=(m == HM // 128 - 1))
            OT = work.tile([128, 2, 512], BF16, tag="OT")
            c2_ap = sm[:, _C_C2 + l:_C_C2 + l + 1]
            nc.vector.tensor_scalar_add(OT[:, 0, :], psoL, c2_ap)
            nc.vector.tensor_scalar_add(OT[:, 1, :], psoR, c2_ap)
            ot_flat = OT.rearrange("p a b -> p (a b)")
            for i in range(NT):
                pst = ps_sc.tile([128, 512], BF16, tag="psg")
                nc.tensor.transpose(pst[:, 0:128],
                                    ot_flat[:, bass.ts(i, 128)], ident_b)
                nc.vector.tensor_add(Hc[:, i, :], pst[:, 0:128], Hc[:, i, :])

        if dump_hc:
            nc.sync.dma_start(dbg[:, :, :], Hc)

        # ---- head: pred = Hc @ Wro^T + bro ----
        wro_s = singles.tile([128, 1], F32R)
        nc.scalar.copy(wro_s, sm[:, _C_WRO:_C_WRO + 1])
        HcT = work.tile([128, TL], F32R, tag="hct")
        for i in range(NT):
            hr = small.tile([128, 128], F32R, tag="hr")
            nc.vector.tensor_copy(hr, Hc[:, i, :])
            psh = ps_sc.tile([128, 512], F32R, tag="psg")
            nc.tensor.transpose(psh[:, 0:128], hr, ident)
            nc.vector.tensor_copy(HcT[:, bass.ts(i, 128)], psh[:, 0:128])
        pred = work.tile([1, TL], F32, tag="pred")
        for j in range(TL // 512):
            psp = ps_sc.tile([1, 512], F32, tag="psg")
            nc.tensor.matmul(psp, wro_s, HcT[:, bass.ts(j, 512)],
                             start=True, stop=True)
            nc.scalar.activation(pred[:, bass.ts(j, 512)], psp,
                                 AF.Identity,
                                 bias=sm[0:1, _C_BRO:_C_BRO + 1], scale=1.0)
        nc.sync.dma_start(out[:, :], pred)

    nc.compile()
    return nc, n_cores


_CACHE = {}


def _get_built(mode="m8"):
    if "m8" not in _CACHE:
        _CACHE["m8"] = _build()
    return _CACHE["m8"]


MODE = "m8"


def _prep_inputs(mode, x, pos_emb, Wq, Wk, Wv, ln1_g, ln1_b, W1, b1, W2, b2,
                 ln2_g, ln2_b, Wro, bro):
    """Host-side shard + fold. Returns list of per-core input dicts."""
    n_cores = 8
    TL = T // 2

    f32 = np.float32
    x = np.asarray(x).astype(np.int32)
    pos_emb = np.asarray(pos_emb, f32)
    Wq, Wk, Wv = (np.asarray(a, f32) for a in (Wq, Wk, Wv))
    ln1_g, ln1_b = np.asarray(ln1_g, f32), np.asarray(ln1_b, f32)
    W1, b1 = np.asarray(W1, f32), np.asarray(b1, f32)
    W2, b2 = np.asarray(W2, f32), np.asarray(b2, f32)
    ln2_g, ln2_b = np.asarray(ln2_g, f32), np.asarray(ln2_b, f32)
    Wro, bro = np.asarray(Wro, f32), np.asarray(bro, f32)

    # fold LN gains into projections; LN betas into the bias columns
    wqkT = np.stack([
        np.stack([(Wq[l] * ln1_g[l][None, :]).T,
                  (Wk[l] * ln1_g[l][None, :]).T]) for l in range(L)])
    wvT = np.stack([(Wv[l] * ln1_g[l][None, :]).T for l in range(L)])
    w1T = np.stack([(W1[l] * ln2_g[l][None, :]).T for l in range(L)])
    w2T = np.stack([W2[l].T for l in range(L)])

    smalls = np.zeros((D, 128), f32)
    for l in range(L):
        smalls[:, _C_CQ + l] = ln1_b[l] @ Wq[l].T
        smalls[:, _C_CV + l] = ln1_b[l] @ Wv[l].T
        c1 = (b1[l] + ln2_b[l] @ W1[l].T).reshape(HM // D, D)
        for m in range(HM // D):
            smalls[:, _C_C1 + 4 * l + m] = c1[m]
        smalls[:, _C_C2 + l] = b2[l]
    smalls[:, _C_WRO] = Wro[0]
    smalls[:, _C_BRO] = float(bro.reshape(-1)[0])

    ar = np.arange(D, dtype=f32)
    mask = (ar < V).astype(f32)
    aconst = np.stack([-0.5 * ar * ar * mask, ar * mask, -0.5 * mask])

    bf16 = ml_dtypes.bfloat16
    common = dict(
        aconst=np.ascontiguousarray(aconst, f32),
        wqkT=np.ascontiguousarray(wqkT.astype(bf16)),
        wvT=np.ascontiguousarray(wvT.astype(bf16)),
        w1T=np.ascontiguousarray(w1T.astype(bf16)),
        w2T=np.ascontiguousarray(w2T.astype(bf16)),
        smalls=np.ascontiguousarray(smalls, f32),
    )
    in_maps = []
    for c in range(n_cores):
        b_idx, half = c // 2, c % 2
        sl = slice(half * TL, (half + 1) * TL)
        xrow = np.zeros((1, TL + 128), np.int32)
        xrow[0, :TL] = x[b_idx, sl]
        in_maps.append(dict(
            common,
            xin=xrow,
            pose=np.ascontiguousarray(pos_emb[sl]),
        ))
    return in_maps


def kernel(**inputs):
    nc, n_cores = _get_built()
    in_maps = _prep_inputs("m8", **inputs)
    res = run_bass_kernel_spmd(nc, in_maps, core_ids=list(range(n_cores)))
    TL = T // 2
    out = np.empty((B, T), np.float32)
    for c in range(n_cores):
        b_idx, half = c // 2, c % 2
        out[b_idx, half * TL:(half + 1) * TL] = res.results[c]["out"][0]
    return out
